# revision 17
# baseline (speedup 1.0000x reference)
"""Causal multi-head attention (B=1, S=4096, D=768, H=12, d_head=64) on 8
Trainium2 NeuronCores.

Sharding: tensor-parallel over heads. 12 heads are mapped onto 16 head-slots
(2 per core); the 4 leftover heads are duplicated onto two slots of the same
core with their W_out rows pre-scaled by 0.5, keeping the SPMD program
uniform across cores. Each core computes Q/K/V projections for its 2 head
slots, causal flash-attention (exp without max-subtraction; softmax
denominator obtained free via an appended ones-column on V), and a partial
row-parallel out-projection. The host sums the 8 partial outputs and adds
b_out (the all-reduce step of the row-parallel out projection).

v2: bf16 data path. The host pre-casts x and all weights to bf16; x is
transposed on the fly by the DMA XBAR (dma transpose), so the PE runs no
transposes at all. Matmuls stream bf16 (1 cycle/row, lower power than
float32r -> less HAM throttling). Scores for the two head slots go to one
2-bank PSUM tile so a single activation instruction exponentiates both
slots. Softmax reciprocal uses the fast approximate DVE op. Per-q-tile
interleaving: project tile t, then attention for tile t, with tile t-1's
out-projection slotted between to hide the softmax-normalize latency.
"""

import sys

sys.path.insert(0, "/opt/trn_rl_repo")

import ml_dtypes
import numpy as np

import concourse.bass as bass
import concourse.tile as tile
from concourse import bacc, mybir
from concourse.bass_utils import run_bass_kernel_spmd

S = 4096
D = 768
HD = 64
P = 128
KC = D // P  # 6 contraction chunks for the projections
QT_W = 512  # query-tile width (psum free dim)
NQT = S // QT_W  # 8 query tiles
NKB = S // P  # 32 key blocks
NEG = -1e30

F32 = mybir.dt.float32
BF16 = mybir.dt.bfloat16
AF = mybir.ActivationFunctionType
BF = ml_dtypes.bfloat16

SLOTS = [(0, 1), (2, 3), (4, 5), (6, 7), (8, 8), (9, 9), (10, 10), (11, 11)]
SCALES = [(1.0, 1.0)] * 4 + [(0.5, 0.5)] * 4

_CACHED_NC = None


def build_nc():
    nc = bacc.Bacc("TRN2", target_bir_lowering=False, debug=False, num_devices=8)

    xt_d = nc.declare_dram_parameter("xt", [P, KC, S], BF16, isOutput=False)
    w_d = nc.declare_dram_parameter("w", [P, KC, 3 * P], BF16, isOutput=False)
    wo_d = nc.declare_dram_parameter("wo", [P, D], BF16, isOutput=False)
    mask_d = nc.declare_dram_parameter("mask", [P, P], F32, isOutput=False)
    out_d = nc.declare_dram_parameter("out", [S, D], F32, isOutput=True)

    with tile.TileContext(nc) as tc:
        with (
            tc.tile_pool(name="const", bufs=1) as const,
            tc.tile_pool(name="big", bufs=1) as big,
        ):
            # ---- constants / staging ----
            warm = const.tile([P, 256], BF16)
            nc.gpsimd.memset(warm[:], 0.0)
            mask_s = const.tile([P, P], F32)
            nc.sync.dma_start(mask_s[:], mask_d[:])
            w = const.tile([P, KC, 3 * P], BF16)
            nc.sync.dma_start(w[:], w_d[:])
            wo = const.tile([P, D], BF16)
            nc.sync.dma_start(wo[:], wo_d[:])

            # qT/kT: packed 2-slot layout straight from the projection PSUM:
            # slot s occupies partitions s*64:(s+1)*64; scores contract K=64.
            xT = big.tile([P, KC, S], BF16)
            qT = big.tile([P, S], BF16)
            kT = big.tile([P, S], BF16)
            # vA: per key block, per slot: [64 v-dims + ones column] so the PV
            # matmul's lhsT [128 keys, 65] yields ctx rows 0:64 and the
            # softmax denominator in row 64.
            vA = big.tile([P, NKB, 2, 65], BF16)
            nc.gpsimd.memset(vA[:, :, :, 64:65], 1.0)
            cT = big.tile([P, S], BF16)
            scratch = const.tile([P, P], F32)

            # xT comes pre-transposed from the host; one DMA per q-tile (all 6
            # chunks) keeps the sync queue's per-DMA issue overhead off the
            # critical path while projections still only wait on their tile.
            for t in range(NQT):
                nc.sync.dma_start(
                    xT[:, :, t * QT_W : (t + 1) * QT_W],
                    xt_d[:, :, t * QT_W : (t + 1) * QT_W],
                )

            with (
                tc.tile_pool(name="pjp", bufs=2, space="PSUM") as pjp,
                tc.tile_pool(name="scp", bufs=2, space="PSUM") as scp,
                tc.tile_pool(name="ctp", bufs=1, space="PSUM") as ctp,
                tc.tile_pool(name="vt", bufs=2) as vtp,
                tc.tile_pool(name="pt", bufs=4) as pt,
                tc.tile_pool(name="sm", bufs=4) as sm,
            ):
                # warm up the PE HAM + preload the Exp table while DMAs run
                for wi in range(48):
                    wps = pjp.tile([P, 256], F32, name="warm_ps", tag="pj")
                    nc.tensor.matmul(
                        wps[:], warm[:, 0:P], warm[:], start=True, stop=True
                    )
                nc.scalar.activation(scratch[:], mask_s[:], AF.Exp, scale=0.125)

                def outproj_st(st):
                    o_stage = sm.tile([P, D], F32, name="o_stage", bufs=3)
                    for nch in range(2):
                        po = pjp.tile([P, QT_W], F32, name="po", tag="pj")
                        nc.tensor.matmul(
                            po[:, : D // 2],
                            cT[:, st * P : (st + 1) * P],
                            wo[:, nch * (D // 2) : (nch + 1) * (D // 2)],
                            start=True,
                            stop=True,
                        )
                        nc.vector.tensor_copy(
                            o_stage[:, nch * (D // 2) : (nch + 1) * (D // 2)],
                            po[:, : D // 2],
                        )
                    nc.sync.dma_start(out_d[st * P : (st + 1) * P, :], o_stage[:])

                def proj_chain(t, pi):
                    tsl = slice(t * QT_W, (t + 1) * QT_W)
                    pj = pjp.tile([P, QT_W], F32, name="pj", tag="pj")
                    for c in range(KC):
                        nc.tensor.matmul(
                            pj[:],
                            w[:, c, pi * P : (pi + 1) * P],
                            xT[:, c, tsl],
                            start=(c == 0),
                            stop=(c == KC - 1),
                        )
                    if pi < 2:
                        nc.vector.tensor_copy((qT, kT)[pi][:, tsl], pj[:])
                        return
                    vt = vtp.tile([P, QT_W], BF16, name="vt")
                    nc.vector.tensor_copy(vt[:], pj[:])
                    # V back to natural [keys, vdims] layout via DMA XBAR.
                    # The XBAR needs a contiguous destination, so land in a
                    # staging tile and split the two slots with DVE copies.
                    for b in range(4):
                        kb = 4 * t + b
                        vN = vtp.tile([P, P], BF16, name="vN", bufs=3)
                        nc.sync.dma_start(
                            vN[:],
                            vt[:, b * P : (b + 1) * P],
                            transpose=True,
                        )
                        for slot in (0, 1):
                            nc.vector.tensor_copy(
                                vA[:, kb, slot, 0:64],
                                vN[:, slot * HD : (slot + 1) * HD],
                            )

                # Two-tile projection lookahead, with proj(t+2)'s chains and
                # outproj(t-1)'s subtiles interleaved INTO tile t's kb loop as
                # PE filler: the activation engine paces the attention inner
                # loop, and without filler the PE's idle windows re-arm the
                # HAM clock gate (K=4/8, half speed). Back-to-back PE work
                # keeps K=8/8.
                for pi in range(3):
                    proj_chain(0, pi)
                for pi in range(3):
                    proj_chain(1, pi)
                for t in range(NQT):
                    tsl = slice(t * QT_W, (t + 1) * QT_W)
                    fillers = []
                    if t + 2 < NQT:
                        fillers += [lambda pi=pi: proj_chain(t + 2, pi) for pi in range(3)]
                    if t > 0:
                        fillers += [lambda st=st: outproj_st(st) for st in range(4 * (t - 1), 4 * t)]
                    # ---- attention for q-tile t ----
                    nkb = 4 * (t + 1)
                    spacing = max(1, nkb // (len(fillers) + 1))
                    ctx = [
                        ctp.tile([P, QT_W], F32, name=f"ctx{s}", tag=f"ctx{s}")
                        for s in (0, 1)
                    ]
                    for kb in range(nkb):
                        r = kb * P - t * QT_W  # diagonal offset
                        r0 = max(0, r)
                        sc2 = scp.tile([P, 2, QT_W], F32, name="sc", tag="sc")
                        for slot in (0, 1):
                            ssl = slice(slot * HD, (slot + 1) * HD)
                            nc.tensor.matmul(
                                sc2[:, slot, r0:QT_W],
                                kT[ssl, kb * P : (kb + 1) * P],
                                qT[ssl, t * QT_W + r0 : (t + 1) * QT_W],
                                start=True,
                                stop=True,
                            )
                        if r >= 0:
                            nc.vector.tensor_tensor(
                                sc2[:, :, r : r + P],
                                sc2[:, :, r : r + P],
                                mask_s[:, None, :].broadcast_to([P, 2, P]),
                                mybir.AluOpType.add,
                            )
                        p2 = pt.tile([P, 2, QT_W], BF16, name="p2")
                        nc.scalar.activation(
                            p2[:, :, r0:QT_W],
                            sc2[:, :, r0:QT_W],
                            AF.Exp,
                            scale=0.125,
                        )
                        for slot in (0, 1):
                            nc.tensor.matmul(
                                ctx[slot][0:65, r0:QT_W],
                                vA[:, kb, slot, :],
                                p2[:, slot, r0:QT_W],
                                start=(kb == 0),
                                stop=(kb == nkb - 1),
                            )
                        if fillers and (kb + 1) % spacing == 0:
                            fillers.pop(0)()
                    # softmax normalization; cT rows 0:64 slot0, 64:128 slot1
                    for slot in (0, 1):
                        # the custom-DVE reciprocal can't read PSUM; stage the
                        # denominator row through SBUF first
                        dsb = sm.tile([1, QT_W], F32, name="dsb")
                        nc.vector.tensor_copy(dsb[:], ctx[slot][64:65, :])
                        lr = sm.tile([1, QT_W], F32, name="lr")
                        nc.vector.reciprocal_approx_fast(lr[:], dsb[:])
                        lb = sm.tile([64, QT_W], F32, name="lb")
                        nc.gpsimd.partition_broadcast(lb[:], lr[0:1, :])
                        nc.vector.tensor_tensor(
                            cT[slot * HD : (slot + 1) * HD, tsl],
                            ctx[slot][0:64, :],
                            lb[:],
                            mybir.AluOpType.mult,
                        )
                    for f in fillers:
                        f()
                for st in range(4 * (NQT - 1), 4 * NQT):
                    outproj_st(st)

    nc.compile()
    return nc


def _host_inputs(x, W_query, W_key, W_value, W_out):
    mask = np.where(
        np.arange(P)[:, None] <= np.arange(P)[None, :], 0.0, NEG
    ).astype(np.float32)
    # host-side transpose: xt[p, c, s] = x[s, c*128 + p]
    xt = np.ascontiguousarray(
        x.astype(BF).T.reshape(KC, P, S).transpose(1, 0, 2)
    )
    in_maps = []
    for core in range(8):
        ha, hb = SLOTS[core]
        sa, sb = SCALES[core]
        ca, cb = slice(ha * HD, (ha + 1) * HD), slice(hb * HD, (hb + 1) * HD)
        # packed per-core projection weights [768, 128] -> [128(p), 6(c), 128]
        def pack(wm):
            sel = np.concatenate([wm[:, ca], wm[:, cb]], axis=1)  # [768, 128]
            return sel.reshape(KC, P, P).transpose(1, 0, 2)  # [p, c, m]

        wq, wk, wv = pack(W_query), pack(W_key), pack(W_value)
        w_all = np.concatenate([wq, wk, wv], axis=2).astype(BF)  # [128, 6, 384]
        wo = np.concatenate([W_out[ca, :] * sa, W_out[cb, :] * sb], axis=0).astype(
            BF
        )
        in_maps.append(
            {
                "xt": xt,
                "w": np.ascontiguousarray(w_all),
                "wo": np.ascontiguousarray(wo),
                "mask": mask,
            }
        )
    return in_maps


def run(x, W_query, W_key, W_value, W_out, b_out, trace=False):
    global _CACHED_NC
    if _CACHED_NC is None:
        _CACHED_NC = build_nc()
    nc = _CACHED_NC
    in_maps = _host_inputs(x, W_query, W_key, W_value, W_out)
    res = run_bass_kernel_spmd(nc, in_maps, core_ids=list(range(8)), trace=trace)
    out = np.zeros((S, D), dtype=np.float32)
    for core in range(8):
        out += res.results[core]["out"]
    out += b_out[None, :].astype(np.float32)
    return out, res


def kernel(x, W_query, W_key, W_value, W_out, b_out):
    x2 = np.asarray(x, dtype=np.float32).reshape(S, D)
    out, _ = run(
        x2,
        np.asarray(W_query, np.float32),
        np.asarray(W_key, np.float32),
        np.asarray(W_value, np.float32),
        np.asarray(W_out, np.float32),
        np.asarray(b_out, np.float32),
    )
    return out.reshape(1, S, D)


# revision 29
# speedup vs baseline: 1.0315x; 1.0315x over previous
"""Causal multi-head attention (B=1, S=4096, D=768, H=12, d_head=64) on 8
Trainium2 NeuronCores.

Sharding: tensor-parallel over heads. 12 heads are mapped onto 16 head-slots
(2 per core); the 4 leftover heads are duplicated onto two slots of the same
core with their W_out rows pre-scaled by 0.5, keeping the SPMD program
uniform across cores. Each core computes Q/K/V projections for its 2 head
slots, causal flash-attention (exp without max-subtraction; softmax
denominator obtained free via an appended ones-column on V), and a partial
row-parallel out-projection. The host sums the 8 partial outputs and adds
b_out (the all-reduce step of the row-parallel out projection).

v2: bf16 data path. The host pre-casts x and all weights to bf16; x is
transposed on the fly by the DMA XBAR (dma transpose), so the PE runs no
transposes at all. Matmuls stream bf16 (1 cycle/row, lower power than
float32r -> less HAM throttling). Scores for the two head slots go to one
2-bank PSUM tile so a single activation instruction exponentiates both
slots. Softmax reciprocal uses the fast approximate DVE op. Per-q-tile
interleaving: project tile t, then attention for tile t, with tile t-1's
out-projection slotted between to hide the softmax-normalize latency.
"""

import sys

sys.path.insert(0, "/opt/trn_rl_repo")

import ml_dtypes
import numpy as np

import concourse.bass as bass
import concourse.tile as tile
from concourse import bacc, mybir
from concourse.bass_utils import run_bass_kernel_spmd

S = 4096
D = 768
HD = 64
P = 128
KC = D // P  # 6 contraction chunks for the projections
QT_W = 512  # query-tile width (psum free dim)
NQT = S // QT_W  # 8 query tiles
NKB = S // P  # 32 key blocks
NEG = -1e30

F32 = mybir.dt.float32
BF16 = mybir.dt.bfloat16
F8 = mybir.dt.float8e4
AF = mybir.ActivationFunctionType
BF = ml_dtypes.bfloat16
DR = mybir.MatmulPerfMode.DoubleRow

SLOTS = [(0, 1), (2, 3), (4, 5), (6, 7), (8, 8), (9, 9), (10, 10), (11, 11)]
SCALES = [(1.0, 1.0)] * 4 + [(0.5, 0.5)] * 4

_CACHED_NC = None


def build_nc():
    nc = bacc.Bacc("TRN2", target_bir_lowering=False, debug=False, num_devices=8)

    xt_d = nc.declare_dram_parameter("xt", [P, KC, S], BF16, isOutput=False)
    w_d = nc.declare_dram_parameter("w", [P, KC, 3 * P], BF16, isOutput=False)
    wo_d = nc.declare_dram_parameter("wo", [P, D], BF16, isOutput=False)
    mask_d = nc.declare_dram_parameter("mask", [P, P], F32, isOutput=False)
    out_d = nc.declare_dram_parameter("out", [S, D], F32, isOutput=True)

    with tile.TileContext(nc) as tc:
        with (
            tc.tile_pool(name="const", bufs=1) as const,
            tc.tile_pool(name="big", bufs=1) as big,
        ):
            # ---- constants / staging ----
            warm = const.tile([P, 256], BF16)
            nc.gpsimd.memset(warm[:], 0.0)
            mask_s = const.tile([P, P], F32)
            nc.sync.dma_start(mask_s[:], mask_d[:])
            w = const.tile([P, KC, 3 * P], BF16)
            nc.sync.dma_start(w[:], w_d[:])
            wo = const.tile([P, D], BF16)
            nc.sync.dma_start(wo[:], wo_d[:])

            # qT/kT: packed 2-slot layout straight from the projection PSUM:
            # slot s occupies partitions s*64:(s+1)*64; scores contract K=64.
            xT = big.tile([P, KC, S], BF16)
            qT = big.tile([P, S], BF16)
            kT = big.tile([P, S], BF16)
            # vA: per key block, per slot: [64 v-dims + ones column] so the PV
            # matmul's lhsT [128 keys, 65] yields ctx rows 0:64 and the
            # softmax denominator in row 64.
            vA = big.tile([P, NKB, 2, 65], BF16)
            nc.gpsimd.memset(vA[:, :, :, 64:65], 1.0)
            cT = big.tile([P, S], BF16)
            scratch = const.tile([P, P], F32)

            # xT comes pre-transposed from the host; one DMA per q-tile (all 6
            # chunks) keeps the sync queue's per-DMA issue overhead off the
            # critical path while projections still only wait on their tile.
            for t in range(NQT):
                nc.sync.dma_start(
                    xT[:, :, t * QT_W : (t + 1) * QT_W],
                    xt_d[:, :, t * QT_W : (t + 1) * QT_W],
                )

            with (
                tc.tile_pool(name="pjp", bufs=2, space="PSUM") as pjp,
                tc.tile_pool(name="scp", bufs=2, space="PSUM") as scp,
                tc.tile_pool(name="ctp", bufs=1, space="PSUM") as ctp,
                tc.tile_pool(name="vt", bufs=2) as vtp,
                tc.tile_pool(name="pt", bufs=4) as pt,
                tc.tile_pool(name="sm", bufs=4) as sm,
            ):
                # warm up the PE HAM + preload the Exp table while DMAs run
                for wi in range(28):
                    wps = pjp.tile([P, 256], F32, name="warm_ps", tag="pj")
                    nc.tensor.matmul(
                        wps[:], warm[:, 0:P], warm[:], start=True, stop=True
                    )
                nc.scalar.activation(scratch[:], mask_s[:], AF.Exp, scale=0.125)

                def outproj_st(st):
                    o_stage = sm.tile([P, D], F32, name="o_stage", bufs=3)
                    for nch in range(2):
                        po = pjp.tile([P, QT_W], F32, name="po", tag="pj")
                        nc.tensor.matmul(
                            po[:, : D // 2],
                            cT[:, st * P : (st + 1) * P],
                            wo[:, nch * (D // 2) : (nch + 1) * (D // 2)],
                            start=True,
                            stop=True,
                        )
                        nc.vector.tensor_copy(
                            o_stage[:, nch * (D // 2) : (nch + 1) * (D // 2)],
                            po[:, : D // 2],
                        )
                    nc.sync.dma_start(out_d[st * P : (st + 1) * P, :], o_stage[:])

                def proj_chain(t, pi):
                    tsl = slice(t * QT_W, (t + 1) * QT_W)
                    pj = pjp.tile([P, QT_W], F32, name="pj", tag="pj")
                    for c in range(KC):
                        nc.tensor.matmul(
                            pj[:],
                            w[:, c, pi * P : (pi + 1) * P],
                            xT[:, c, tsl],
                            start=(c == 0),
                            stop=(c == KC - 1),
                        )
                    if pi < 2:
                        nc.vector.tensor_copy((qT, kT)[pi][:, tsl], pj[:])
                        return
                    vt = vtp.tile([P, QT_W], BF16, name="vt")
                    nc.vector.tensor_copy(vt[:], pj[:])
                    # V back to natural [keys, vdims] layout via DMA XBAR.
                    # The XBAR needs a contiguous destination, so land in a
                    # staging tile and split the two slots with DVE copies.
                    for b in range(4):
                        kb = 4 * t + b
                        vN = vtp.tile([P, P], BF16, name="vN", bufs=3)
                        nc.sync.dma_start(
                            vN[:],
                            vt[:, b * P : (b + 1) * P],
                            transpose=True,
                        )
                        for slot in (0, 1):
                            nc.vector.tensor_copy(
                                vA[:, kb, slot, 0:64],
                                vN[:, slot * HD : (slot + 1) * HD],
                            )

                # Two-tile projection lookahead, with proj(t+2)'s chains and
                # outproj(t-1)'s subtiles interleaved INTO tile t's kb loop as
                # PE filler: the activation engine paces the attention inner
                # loop, and without filler the PE's idle windows re-arm the
                # HAM clock gate (K=4/8, half speed). Back-to-back PE work
                # keeps K=8/8.
                for pi in range(3):
                    proj_chain(0, pi)
                for pi in range(3):
                    proj_chain(1, pi)
                for t in range(NQT):
                    tsl = slice(t * QT_W, (t + 1) * QT_W)
                    fillers = []
                    if t + 2 < NQT:
                        fillers += [lambda pi=pi: proj_chain(t + 2, pi) for pi in range(3)]
                    if t > 0:
                        fillers += [lambda st=st: outproj_st(st) for st in range(4 * (t - 1), 4 * t)]
                    # ---- attention for q-tile t ----
                    nkb = 4 * (t + 1)
                    spacing = 10**9  # fillers run after the tile's normalize
                    ctx = [
                        ctp.tile([P, QT_W], F32, name=f"ctx{s}", tag=f"ctx{s}")
                        for s in (0, 1)
                    ]
                    for kb in range(nkb):
                        r = kb * P - t * QT_W  # diagonal offset
                        r0 = max(0, r)
                        sc2 = scp.tile([P, 2, QT_W], F32, name="sc", tag="sc")
                        for slot in (0, 1):
                            ssl = slice(slot * HD, (slot + 1) * HD)
                            nc.tensor.matmul(
                                sc2[:, slot, r0:QT_W],
                                kT[ssl, kb * P : (kb + 1) * P],
                                qT[ssl, t * QT_W + r0 : (t + 1) * QT_W],
                                start=True,
                                stop=True,
                            )
                        if r >= 0:
                            nc.vector.tensor_tensor(
                                sc2[:, :, r : r + P],
                                sc2[:, :, r : r + P],
                                mask_s[:, None, :].broadcast_to([P, 2, P]),
                                mybir.AluOpType.add,
                            )
                        p2 = pt.tile([P, 2, QT_W], BF16, name="p2")
                        nc.scalar.activation(
                            p2[:, :, r0:QT_W],
                            sc2[:, :, r0:QT_W],
                            AF.Exp,
                            scale=0.125,
                        )
                        for slot in (0, 1):
                            nc.tensor.matmul(
                                ctx[slot][0:65, r0:QT_W],
                                vA[:, kb, slot, :],
                                p2[:, slot, r0:QT_W],
                                start=(kb == 0),
                                stop=(kb == nkb - 1),
                            )
                        if fillers and (kb + 1) % spacing == 0:
                            fillers.pop(0)()
                    # softmax normalization; cT rows 0:64 slot0, 64:128 slot1
                    for slot in (0, 1):
                        # the custom-DVE reciprocal can't read PSUM; stage the
                        # denominator row through SBUF first
                        dsb = sm.tile([1, QT_W], F32, name="dsb")
                        nc.vector.tensor_copy(dsb[:], ctx[slot][64:65, :])
                        lr = sm.tile([1, QT_W], F32, name="lr")
                        nc.vector.reciprocal_approx_fast(lr[:], dsb[:])
                        lb = sm.tile([64, QT_W], F32, name="lb")
                        nc.gpsimd.partition_broadcast(lb[:], lr[0:1, :])
                        nc.vector.tensor_tensor(
                            cT[slot * HD : (slot + 1) * HD, tsl],
                            ctx[slot][0:64, :],
                            lb[:],
                            mybir.AluOpType.mult,
                        )
                    for f in fillers:
                        f()
                for st in range(4 * (NQT - 1), 4 * NQT):
                    outproj_st(st)

    nc.compile()
    return nc


def _host_inputs(x, W_query, W_key, W_value, W_out):
    mask = np.where(
        np.arange(P)[:, None] <= np.arange(P)[None, :], 0.0, NEG
    ).astype(np.float32)
    # host-side transpose: xt[p, c, s] = x[s, c*128 + p]
    xt = np.ascontiguousarray(
        x.astype(BF).T.reshape(KC, P, S).transpose(1, 0, 2)
    )
    in_maps = []
    for core in range(8):
        ha, hb = SLOTS[core]
        sa, sb = SCALES[core]
        ca, cb = slice(ha * HD, (ha + 1) * HD), slice(hb * HD, (hb + 1) * HD)
        # packed per-core projection weights [768, 128] -> [128(p), 6(c), 128]
        def pack(wm):
            sel = np.concatenate([wm[:, ca], wm[:, cb]], axis=1)  # [768, 128]
            return sel.reshape(KC, P, P).transpose(1, 0, 2)  # [p, c, m]

        wq, wk, wv = pack(W_query), pack(W_key), pack(W_value)
        w_all = np.concatenate([wq, wk, wv], axis=2).astype(BF)  # [128, 6, 384]
        wo = np.concatenate([W_out[ca, :] * sa, W_out[cb, :] * sb], axis=0).astype(
            BF
        )
        in_maps.append(
            {
                "xt": xt,
                "w": np.ascontiguousarray(w_all),
                "wo": np.ascontiguousarray(wo),
                "mask": mask,
            }
        )
    return in_maps


def run(x, W_query, W_key, W_value, W_out, b_out, trace=False):
    global _CACHED_NC
    if _CACHED_NC is None:
        _CACHED_NC = build_nc()
    nc = _CACHED_NC
    in_maps = _host_inputs(x, W_query, W_key, W_value, W_out)
    res = run_bass_kernel_spmd(nc, in_maps, core_ids=list(range(8)), trace=trace)
    out = np.zeros((S, D), dtype=np.float32)
    for core in range(8):
        out += res.results[core]["out"]
    out += b_out[None, :].astype(np.float32)
    return out, res


def kernel(x, W_query, W_key, W_value, W_out, b_out):
    x2 = np.asarray(x, dtype=np.float32).reshape(S, D)
    out, _ = run(
        x2,
        np.asarray(W_query, np.float32),
        np.asarray(W_key, np.float32),
        np.asarray(W_value, np.float32),
        np.asarray(W_out, np.float32),
        np.asarray(b_out, np.float32),
    )
    return out.reshape(1, S, D)


# revision 34
# speedup vs baseline: 1.0541x; 1.0219x over previous
"""Causal multi-head attention (B=1, S=4096, D=768, H=12, d_head=64) on 8
Trainium2 NeuronCores.

Sharding: tensor-parallel over heads. 12 heads are mapped onto 16 head-slots
(2 per core); the 4 leftover heads are duplicated onto two slots of the same
core with their W_out rows pre-scaled by 0.5, keeping the SPMD program
uniform across cores. Each core computes Q/K/V projections for its 2 head
slots, causal flash-attention (exp without max-subtraction; softmax
denominator obtained free via an appended ones-column on V), and a partial
row-parallel out-projection. The host sums the 8 partial outputs and adds
b_out (the all-reduce step of the row-parallel out projection).

v2: bf16 data path. The host pre-casts x and all weights to bf16; x is
transposed on the fly by the DMA XBAR (dma transpose), so the PE runs no
transposes at all. Matmuls stream bf16 (1 cycle/row, lower power than
float32r -> less HAM throttling). Scores for the two head slots go to one
2-bank PSUM tile so a single activation instruction exponentiates both
slots. Softmax reciprocal uses the fast approximate DVE op. Per-q-tile
interleaving: project tile t, then attention for tile t, with tile t-1's
out-projection slotted between to hide the softmax-normalize latency.
"""

import sys

sys.path.insert(0, "/opt/trn_rl_repo")

import ml_dtypes
import numpy as np

import concourse.bass as bass
import concourse.tile as tile
from concourse import bacc, mybir
from concourse.bass_utils import run_bass_kernel_spmd

S = 4096
D = 768
HD = 64
P = 128
KC = D // P  # 6 contraction chunks for the projections
QT_W = 512  # query-tile width (psum free dim)
NQT = S // QT_W  # 8 query tiles
NKB = S // P  # 32 key blocks
NEG = -1e30

F32 = mybir.dt.float32
BF16 = mybir.dt.bfloat16
F8 = mybir.dt.float8e4
AF = mybir.ActivationFunctionType
BF = ml_dtypes.bfloat16
DR = mybir.MatmulPerfMode.DoubleRow

SLOTS = [(0, 1), (2, 3), (4, 5), (6, 7), (8, 8), (9, 9), (10, 10), (11, 11)]
SCALES = [(1.0, 1.0)] * 4 + [(0.5, 0.5)] * 4

_CACHED_NC = None


def build_nc():
    nc = bacc.Bacc("TRN2", target_bir_lowering=False, debug=False, num_devices=8)

    xt_d = nc.declare_dram_parameter("xt", [P, KC, S], BF16, isOutput=False)
    w_d = nc.declare_dram_parameter("w", [P, KC, 3 * P], BF16, isOutput=False)
    wo_d = nc.declare_dram_parameter("wo", [P, D], BF16, isOutput=False)
    mask_d = nc.declare_dram_parameter("mask", [P, P], F32, isOutput=False)
    out_d = nc.declare_dram_parameter("out", [S, D], BF16, isOutput=True)

    with tile.TileContext(nc) as tc:
        with (
            tc.tile_pool(name="const", bufs=1) as const,
            tc.tile_pool(name="big", bufs=1) as big,
        ):
            # ---- constants / staging ----
            warm = const.tile([P, 256], BF16)
            nc.gpsimd.memset(warm[:], 0.0)
            mask_s = const.tile([P, P], F32)
            nc.sync.dma_start(mask_s[:], mask_d[:])
            w = const.tile([P, KC, 3 * P], BF16)
            nc.sync.dma_start(w[:], w_d[:])
            wo = const.tile([P, D], BF16)
            nc.sync.dma_start(wo[:], wo_d[:])

            # qT/kT: packed 2-slot layout straight from the projection PSUM:
            # slot s occupies partitions s*64:(s+1)*64; scores contract K=64.
            xT = big.tile([P, KC, S], BF16)
            qT = big.tile([P, S], BF16)
            kT = big.tile([P, S], BF16)
            # vA: per key block, per slot: [64 v-dims + ones column] so the PV
            # matmul's lhsT [128 keys, 65] yields ctx rows 0:64 and the
            # softmax denominator in row 64.
            vA = big.tile([P, NKB, 2, 65], BF16)
            nc.gpsimd.memset(vA[:, :, :, 64:65], 1.0)
            cT = big.tile([P, S], BF16)
            scratch = const.tile([P, P], F32)

            # xT comes pre-transposed from the host; one DMA per q-tile (all 6
            # chunks) keeps the sync queue's per-DMA issue overhead off the
            # critical path while projections still only wait on their tile.
            for t in range(NQT):
                nc.sync.dma_start(
                    xT[:, :, t * QT_W : (t + 1) * QT_W],
                    xt_d[:, :, t * QT_W : (t + 1) * QT_W],
                )

            with (
                tc.tile_pool(name="pjp", bufs=2, space="PSUM") as pjp,
                tc.tile_pool(name="scp", bufs=2, space="PSUM") as scp,
                tc.tile_pool(name="ctp", bufs=1, space="PSUM") as ctp,
                tc.tile_pool(name="vt", bufs=2) as vtp,
                tc.tile_pool(name="pt", bufs=4) as pt,
                tc.tile_pool(name="sm", bufs=4) as sm,
            ):
                # warm up the PE HAM + preload the Exp table while DMAs run
                for wi in range(28):
                    wps = pjp.tile([P, 256], F32, name="warm_ps", tag="pj")
                    nc.tensor.matmul(
                        wps[:], warm[:, 0:P], warm[:], start=True, stop=True
                    )
                nc.scalar.activation(scratch[:], mask_s[:], AF.Exp, scale=0.125)

                def outproj_st(st):
                    o_stage = sm.tile([P, D], BF16, name="o_stage", bufs=3)
                    for nch in range(2):
                        po = pjp.tile([P, QT_W], F32, name="po", tag="pj")
                        nc.tensor.matmul(
                            po[:, : D // 2],
                            cT[:, st * P : (st + 1) * P],
                            wo[:, nch * (D // 2) : (nch + 1) * (D // 2)],
                            start=True,
                            stop=True,
                        )
                        nc.vector.tensor_copy(
                            o_stage[:, nch * (D // 2) : (nch + 1) * (D // 2)],
                            po[:, : D // 2],
                        )
                    nc.sync.dma_start(out_d[st * P : (st + 1) * P, :], o_stage[:])

                def proj_chain(t, pi):
                    tsl = slice(t * QT_W, (t + 1) * QT_W)
                    pj = pjp.tile([P, QT_W], F32, name="pj", tag="pj")
                    for c in range(KC):
                        nc.tensor.matmul(
                            pj[:],
                            w[:, c, pi * P : (pi + 1) * P],
                            xT[:, c, tsl],
                            start=(c == 0),
                            stop=(c == KC - 1),
                        )
                    if pi < 2:
                        nc.vector.tensor_copy((qT, kT)[pi][:, tsl], pj[:])
                        return
                    vt = vtp.tile([P, QT_W], BF16, name="vt")
                    nc.vector.tensor_copy(vt[:], pj[:])
                    # V back to natural [keys, vdims] layout via DMA XBAR.
                    # The XBAR needs a contiguous destination, so land in a
                    # staging tile and split the two slots with DVE copies.
                    for b in range(4):
                        kb = 4 * t + b
                        vN = vtp.tile([P, P], BF16, name="vN", bufs=3)
                        nc.sync.dma_start(
                            vN[:],
                            vt[:, b * P : (b + 1) * P],
                            transpose=True,
                        )
                        for slot in (0, 1):
                            nc.vector.tensor_copy(
                                vA[:, kb, slot, 0:64],
                                vN[:, slot * HD : (slot + 1) * HD],
                            )

                # Two-tile projection lookahead, with proj(t+2)'s chains and
                # outproj(t-1)'s subtiles interleaved INTO tile t's kb loop as
                # PE filler: the activation engine paces the attention inner
                # loop, and without filler the PE's idle windows re-arm the
                # HAM clock gate (K=4/8, half speed). Back-to-back PE work
                # keeps K=8/8.
                for pi in range(3):
                    proj_chain(0, pi)
                for pi in range(3):
                    proj_chain(1, pi)
                for t in range(NQT):
                    tsl = slice(t * QT_W, (t + 1) * QT_W)
                    fillers = []
                    if t + 2 < NQT:
                        fillers += [lambda pi=pi: proj_chain(t + 2, pi) for pi in range(3)]
                    if t > 0:
                        fillers += [lambda st=st: outproj_st(st) for st in range(4 * (t - 1), 4 * t)]
                    # ---- attention for q-tile t ----
                    nkb = 4 * (t + 1)
                    spacing = 10**9  # fillers run after the tile's normalize
                    ctx = [
                        ctp.tile([P, QT_W], F32, name=f"ctx{s}", tag=f"ctx{s}")
                        for s in (0, 1)
                    ]
                    for kb in range(nkb):
                        r = kb * P - t * QT_W  # diagonal offset
                        r0 = max(0, r)
                        sc2 = scp.tile([P, 2, QT_W], F32, name="sc", tag="sc")
                        for slot in (0, 1):
                            ssl = slice(slot * HD, (slot + 1) * HD)
                            nc.tensor.matmul(
                                sc2[:, slot, r0:QT_W],
                                kT[ssl, kb * P : (kb + 1) * P],
                                qT[ssl, t * QT_W + r0 : (t + 1) * QT_W],
                                start=True,
                                stop=True,
                            )
                        if r >= 0:
                            nc.vector.tensor_tensor(
                                sc2[:, :, r : r + P],
                                sc2[:, :, r : r + P],
                                mask_s[:, None, :].broadcast_to([P, 2, P]),
                                mybir.AluOpType.add,
                            )
                        p2 = pt.tile([P, 2, QT_W], BF16, name="p2")
                        nc.scalar.activation(
                            p2[:, :, r0:QT_W],
                            sc2[:, :, r0:QT_W],
                            AF.Exp,
                            scale=0.125,
                        )
                        for slot in (0, 1):
                            nc.tensor.matmul(
                                ctx[slot][0:65, r0:QT_W],
                                vA[:, kb, slot, :],
                                p2[:, slot, r0:QT_W],
                                start=(kb == 0),
                                stop=(kb == nkb - 1),
                            )
                        if fillers and (kb + 1) % spacing == 0:
                            fillers.pop(0)()
                    # softmax normalization; cT rows 0:64 slot0, 64:128 slot1
                    for slot in (0, 1):
                        # the custom-DVE reciprocal can't read PSUM; stage the
                        # denominator row through SBUF first
                        dsb = sm.tile([1, QT_W], F32, name="dsb")
                        nc.vector.tensor_copy(dsb[:], ctx[slot][64:65, :])
                        lr = sm.tile([1, QT_W], F32, name="lr")
                        nc.vector.reciprocal_approx_fast(lr[:], dsb[:])
                        lb = sm.tile([64, QT_W], F32, name="lb")
                        nc.gpsimd.partition_broadcast(lb[:], lr[0:1, :])
                        nc.vector.tensor_tensor(
                            cT[slot * HD : (slot + 1) * HD, tsl],
                            ctx[slot][0:64, :],
                            lb[:],
                            mybir.AluOpType.mult,
                        )
                    for f in fillers:
                        f()
                for st in range(4 * (NQT - 1), 4 * NQT):
                    outproj_st(st)

    nc.compile()
    return nc


def _host_inputs(x, W_query, W_key, W_value, W_out):
    mask = np.where(
        np.arange(P)[:, None] <= np.arange(P)[None, :], 0.0, NEG
    ).astype(np.float32)
    # host-side transpose: xt[p, c, s] = x[s, c*128 + p]
    xt = np.ascontiguousarray(
        x.astype(BF).T.reshape(KC, P, S).transpose(1, 0, 2)
    )
    in_maps = []
    for core in range(8):
        ha, hb = SLOTS[core]
        sa, sb = SCALES[core]
        ca, cb = slice(ha * HD, (ha + 1) * HD), slice(hb * HD, (hb + 1) * HD)
        # packed per-core projection weights [768, 128] -> [128(p), 6(c), 128]
        def pack(wm):
            sel = np.concatenate([wm[:, ca], wm[:, cb]], axis=1)  # [768, 128]
            return sel.reshape(KC, P, P).transpose(1, 0, 2)  # [p, c, m]

        wq, wk, wv = pack(W_query), pack(W_key), pack(W_value)
        w_all = np.concatenate([wq, wk, wv], axis=2).astype(BF)  # [128, 6, 384]
        wo = np.concatenate([W_out[ca, :] * sa, W_out[cb, :] * sb], axis=0).astype(
            BF
        )
        in_maps.append(
            {
                "xt": xt,
                "w": np.ascontiguousarray(w_all),
                "wo": np.ascontiguousarray(wo),
                "mask": mask,
            }
        )
    return in_maps


def run(x, W_query, W_key, W_value, W_out, b_out, trace=False):
    global _CACHED_NC
    if _CACHED_NC is None:
        _CACHED_NC = build_nc()
    nc = _CACHED_NC
    in_maps = _host_inputs(x, W_query, W_key, W_value, W_out)
    res = run_bass_kernel_spmd(nc, in_maps, core_ids=list(range(8)), trace=trace)
    out = np.zeros((S, D), dtype=np.float32)
    for core in range(8):
        out += np.asarray(res.results[core]["out"], dtype=np.float32)
    out += b_out[None, :].astype(np.float32)
    return out, res


def kernel(x, W_query, W_key, W_value, W_out, b_out):
    x2 = np.asarray(x, dtype=np.float32).reshape(S, D)
    out, _ = run(
        x2,
        np.asarray(W_query, np.float32),
        np.asarray(W_key, np.float32),
        np.asarray(W_value, np.float32),
        np.asarray(W_out, np.float32),
        np.asarray(b_out, np.float32),
    )
    return out.reshape(1, S, D)


# revision 40
# speedup vs baseline: 1.0607x; 1.0062x over previous
"""Causal multi-head attention (B=1, S=4096, D=768, H=12, d_head=64) on 8
Trainium2 NeuronCores.

Sharding: tensor-parallel over heads. 12 heads are mapped onto 16 head-slots
(2 per core); the 4 leftover heads are duplicated onto two slots of the same
core with their W_out rows pre-scaled by 0.5, keeping the SPMD program
uniform across cores. Each core computes Q/K/V projections for its 2 head
slots, causal flash-attention (exp without max-subtraction; softmax
denominator obtained free via an appended ones-column on V), and a partial
row-parallel out-projection. The host sums the 8 partial outputs and adds
b_out (the all-reduce step of the row-parallel out projection).

v2: bf16 data path. The host pre-casts x and all weights to bf16; x is
transposed on the fly by the DMA XBAR (dma transpose), so the PE runs no
transposes at all. Matmuls stream bf16 (1 cycle/row, lower power than
float32r -> less HAM throttling). Scores for the two head slots go to one
2-bank PSUM tile so a single activation instruction exponentiates both
slots. Softmax reciprocal uses the fast approximate DVE op. Per-q-tile
interleaving: project tile t, then attention for tile t, with tile t-1's
out-projection slotted between to hide the softmax-normalize latency.
"""

import sys

sys.path.insert(0, "/opt/trn_rl_repo")

import ml_dtypes
import numpy as np

import concourse.bass as bass
import concourse.tile as tile
from concourse import bacc, mybir
from concourse.bass_utils import run_bass_kernel_spmd

S = 4096
D = 768
HD = 64
P = 128
KC = D // P  # 6 contraction chunks for the projections
QT_W = 512  # query-tile width (psum free dim)
NQT = S // QT_W  # 8 query tiles
NKB = S // P  # 32 key blocks
NEG = -1e30

F32 = mybir.dt.float32
BF16 = mybir.dt.bfloat16
F8 = mybir.dt.float8e4
AF = mybir.ActivationFunctionType
BF = ml_dtypes.bfloat16
DR = mybir.MatmulPerfMode.DoubleRow

SLOTS = [(0, 1), (2, 3), (4, 5), (6, 7), (8, 8), (9, 9), (10, 10), (11, 11)]
SCALES = [(1.0, 1.0)] * 4 + [(0.5, 0.5)] * 4

_CACHED_NC = None


def build_nc():
    nc = bacc.Bacc("TRN2", target_bir_lowering=False, debug=False, num_devices=8)

    xt_d = nc.declare_dram_parameter("xt", [P, KC, S], BF16, isOutput=False)
    w_d = nc.declare_dram_parameter("w", [P, KC, 3 * P], BF16, isOutput=False)
    wo_d = nc.declare_dram_parameter("wo", [P, D], BF16, isOutput=False)
    mask_d = nc.declare_dram_parameter("mask", [P, P], F32, isOutput=False)
    out_d = nc.declare_dram_parameter("out", [S, D], BF16, isOutput=True)

    with tile.TileContext(nc) as tc:
        with (
            tc.tile_pool(name="const", bufs=1) as const,
            tc.tile_pool(name="big", bufs=1) as big,
        ):
            # ---- constants / staging ----
            warm = const.tile([P, 256], BF16)
            nc.gpsimd.memset(warm[:], 0.0)
            mask_s = const.tile([P, P], F32)
            nc.sync.dma_start(mask_s[:], mask_d[:])
            w = const.tile([P, KC, 3 * P], BF16)
            nc.sync.dma_start(w[:], w_d[:])
            wo = const.tile([P, D], BF16)
            nc.sync.dma_start(wo[:], wo_d[:])

            # qT/kT: packed 2-slot layout straight from the projection PSUM:
            # slot s occupies partitions s*64:(s+1)*64; scores contract K=64.
            xT = big.tile([P, KC, S], BF16)
            qT = big.tile([P, S], BF16)
            kT = big.tile([P, S], BF16)
            # vA: per key block, per slot: [64 v-dims + ones column] so the PV
            # matmul's lhsT [128 keys, 65] yields ctx rows 0:64 and the
            # softmax denominator in row 64.
            vA = big.tile([P, NKB, 2, 65], BF16)
            nc.gpsimd.memset(vA[:, :, :, 64:65], 1.0)
            cT = big.tile([P, S], BF16)
            scratch = const.tile([P, P], F32)

            # xT comes pre-transposed from the host; one DMA per q-tile (all 6
            # chunks) keeps the sync queue's per-DMA issue overhead off the
            # critical path while projections still only wait on their tile.
            for t in range(NQT):
                nc.sync.dma_start(
                    xT[:, :, t * QT_W : (t + 1) * QT_W],
                    xt_d[:, :, t * QT_W : (t + 1) * QT_W],
                )

            with (
                tc.tile_pool(name="pjp", bufs=2, space="PSUM") as pjp,
                tc.tile_pool(name="scp", bufs=2, space="PSUM") as scp,
                tc.tile_pool(name="ctp", bufs=1, space="PSUM") as ctp,
                tc.tile_pool(name="vt", bufs=2) as vtp,
                tc.tile_pool(name="pt", bufs=4) as pt,
                tc.tile_pool(name="sm", bufs=4) as sm,
            ):
                # warm up the PE HAM + preload the Exp table while DMAs run
                for wi in range(28):
                    wps = pjp.tile([P, 256], F32, name="warm_ps", tag="pj")
                    nc.tensor.matmul(
                        wps[:], warm[:, 0:P], warm[:], start=True, stop=True
                    )
                nc.scalar.activation(scratch[:], mask_s[:], AF.Exp, scale=0.125)

                def outproj_st(st):
                    o_stage = sm.tile([P, D], BF16, name="o_stage", bufs=3)
                    for nch in range(2):
                        po = pjp.tile([P, QT_W], F32, name="po", tag="pj")
                        nc.tensor.matmul(
                            po[:, : D // 2],
                            cT[:, st * P : (st + 1) * P],
                            wo[:, nch * (D // 2) : (nch + 1) * (D // 2)],
                            start=True,
                            stop=True,
                        )
                        nc.vector.tensor_copy(
                            o_stage[:, nch * (D // 2) : (nch + 1) * (D // 2)],
                            po[:, : D // 2],
                        )
                    nc.sync.dma_start(out_d[st * P : (st + 1) * P, :], o_stage[:])

                def proj_chain(t, pi):
                    tsl = slice(t * QT_W, (t + 1) * QT_W)
                    pj = pjp.tile([P, QT_W], F32, name="pj", tag="pj")
                    for c in range(KC):
                        nc.tensor.matmul(
                            pj[:],
                            w[:, c, pi * P : (pi + 1) * P],
                            xT[:, c, tsl],
                            start=(c == 0),
                            stop=(c == KC - 1),
                        )
                    if pi < 2:
                        nc.vector.tensor_copy((qT, kT)[pi][:, tsl], pj[:])
                        return
                    vt = vtp.tile([P, QT_W], BF16, name="vt")
                    nc.vector.tensor_copy(vt[:], pj[:])
                    # V back to natural [keys, vdims] layout via DMA XBAR.
                    # The XBAR needs a contiguous destination, so land in a
                    # staging tile and split the two slots with DVE copies.
                    for b in range(4):
                        kb = 4 * t + b
                        vN = vtp.tile([P, P], BF16, name="vN", bufs=3)
                        nc.sync.dma_start(
                            vN[:],
                            vt[:, b * P : (b + 1) * P],
                            transpose=True,
                        )
                        for slot in (0, 1):
                            nc.vector.tensor_copy(
                                vA[:, kb, slot, 0:64],
                                vN[:, slot * HD : (slot + 1) * HD],
                            )

                # Two-tile projection lookahead, with proj(t+2)'s chains and
                # outproj(t-1)'s subtiles interleaved INTO tile t's kb loop as
                # PE filler: the activation engine paces the attention inner
                # loop, and without filler the PE's idle windows re-arm the
                # HAM clock gate (K=4/8, half speed). Back-to-back PE work
                # keeps K=8/8.
                for pi in range(3):
                    proj_chain(0, pi)
                for pi in range(3):
                    proj_chain(1, pi)
                for t in range(NQT):
                    tsl = slice(t * QT_W, (t + 1) * QT_W)
                    fillers = []
                    if t + 2 < NQT:
                        fillers += [lambda pi=pi: proj_chain(t + 2, pi) for pi in range(3)]
                    if t > 0:
                        fillers += [lambda st=st: outproj_st(st) for st in range(4 * (t - 1), 4 * t)]
                    # ---- attention for q-tile t ----
                    nkb = 4 * (t + 1)
                    spacing = 10**9  # fillers run after the tile's normalize
                    ctx = [
                        ctp.tile([P, QT_W], F32, name=f"ctx{s}", tag=f"ctx{s}")
                        for s in (0, 1)
                    ]
                    for kb in range(nkb):
                        r = kb * P - t * QT_W  # diagonal offset
                        r0 = max(0, r)
                        sc2 = scp.tile([P, 2, QT_W], F32, name="sc", tag="sc")
                        for slot in (0, 1):
                            ssl = slice(slot * HD, (slot + 1) * HD)
                            nc.tensor.matmul(
                                sc2[:, slot, r0:QT_W],
                                kT[ssl, kb * P : (kb + 1) * P],
                                qT[ssl, t * QT_W + r0 : (t + 1) * QT_W],
                                start=True,
                                stop=True,
                            )
                        if r >= 0:
                            nc.vector.tensor_tensor(
                                sc2[:, :, r : r + P],
                                sc2[:, :, r : r + P],
                                mask_s[:, None, :].broadcast_to([P, 2, P]),
                                mybir.AluOpType.add,
                            )
                        p2 = pt.tile([P, 2, QT_W], BF16, name="p2")
                        nc.scalar.activation(
                            p2[:, :, r0:QT_W],
                            sc2[:, :, r0:QT_W],
                            AF.Exp,
                            scale=0.125,
                        )
                        for slot in (0, 1):
                            nc.tensor.matmul(
                                ctx[slot][0:65, r0:QT_W],
                                vA[:, kb, slot, :],
                                p2[:, slot, r0:QT_W],
                                start=(kb == 0),
                                stop=(kb == nkb - 1),
                            )
                        if fillers and (kb + 1) % spacing == 0:
                            fillers.pop(0)()
                    # softmax normalization; cT rows 0:64 slot0, 64:128 slot1
                    for slot in (0, 1):
                        # the custom-DVE reciprocal can't read PSUM; stage the
                        # denominator row through SBUF first
                        dsb = sm.tile([1, QT_W], F32, name="dsb")
                        nc.vector.tensor_copy(dsb[:], ctx[slot][64:65, :])
                        lr = sm.tile([1, QT_W], F32, name="lr")
                        nc.vector.reciprocal_approx_fast(lr[:], dsb[:])
                        lb = sm.tile([64, QT_W], F32, name="lb")
                        nc.gpsimd.partition_broadcast(lb[:], lr[0:1, :])
                        nc.vector.tensor_tensor(
                            cT[slot * HD : (slot + 1) * HD, tsl],
                            ctx[slot][0:64, :],
                            lb[:],
                            mybir.AluOpType.mult,
                        )
                    for f in fillers:
                        f()
                for st in range(4 * (NQT - 1), 4 * NQT):
                    outproj_st(st)

    nc.compile()
    return nc


def _host_inputs(x, W_query, W_key, W_value, W_out):
    mask = np.where(
        np.arange(P)[:, None] <= np.arange(P)[None, :], 0.0, NEG
    ).astype(np.float32)
    # host-side transpose: xt[p, c, s] = x[s, c*128 + p]
    xt = np.ascontiguousarray(
        x.astype(BF).T.reshape(KC, P, S).transpose(1, 0, 2)
    )
    in_maps = []
    for core in range(8):
        ha, hb = SLOTS[core]
        sa, sb = SCALES[core]
        ca, cb = slice(ha * HD, (ha + 1) * HD), slice(hb * HD, (hb + 1) * HD)
        # packed per-core projection weights [768, 128] -> [128(p), 6(c), 128]
        def pack(wm):
            sel = np.concatenate([wm[:, ca], wm[:, cb]], axis=1)  # [768, 128]
            return sel.reshape(KC, P, P).transpose(1, 0, 2)  # [p, c, m]

        wq, wk, wv = pack(W_query), pack(W_key), pack(W_value)
        w_all = np.concatenate([wq, wk, wv], axis=2).astype(BF)  # [128, 6, 384]
        wo = np.concatenate([W_out[ca, :] * sa, W_out[cb, :] * sb], axis=0).astype(
            BF
        )
        in_maps.append(
            {
                "xt": xt,
                "w": np.ascontiguousarray(w_all),
                "wo": np.ascontiguousarray(wo),
                "mask": mask,
            }
        )
    return in_maps


def run(x, W_query, W_key, W_value, W_out, b_out, trace=False):
    global _CACHED_NC
    if _CACHED_NC is None:
        _CACHED_NC = build_nc()
    nc = _CACHED_NC
    in_maps = _host_inputs(x, W_query, W_key, W_value, W_out)
    res = run_bass_kernel_spmd(nc, in_maps, core_ids=list(range(8)), trace=trace)
    out = np.zeros((S, D), dtype=np.float32)
    for core in range(8):
        out += np.asarray(res.results[core]["out"], dtype=np.float32)
    out += b_out[None, :].astype(np.float32)
    return out, res


def kernel(x, W_query, W_key, W_value, W_out, b_out):
    x2 = np.asarray(x, dtype=np.float32).reshape(S, D)
    out, _ = run(
        x2,
        np.asarray(W_query, np.float32),
        np.asarray(W_key, np.float32),
        np.asarray(W_value, np.float32),
        np.asarray(W_out, np.float32),
        np.asarray(b_out, np.float32),
    )
    return out.reshape(1, S, D)
